# revision 1
# baseline (speedup 1.0000x reference)
"""DeepEMD episode loss kernel for Trainium2 (8 NeuronCores, data-parallel over episodes).

Algorithm (per core = one episode):
  - S[q,p,m,n] = cosine-sim over channels of centered features  (PE matmuls, fp32r)
  - entropic-OT via Sinkhorn in scaling form: u = a/(Kv), v = b/(K^T u),
    K = exp((S-1)/eps).  8 iterations match the 100-iter log-domain
    reference to ~1e-8 relative (geometric convergence).
  - logits = T * <S, u K v>;  per-query CE returned, mean taken on host.
"""

import numpy as np
from contextlib import ExitStack

import concourse.bass as bass
import concourse.bacc as bacc
import concourse.tile as tile
from concourse import mybir
from concourse.bass_utils import run_bass_kernel_spmd

F32 = mybir.dt.float32
BF16 = mybir.dt.bfloat16
F16 = mybir.dt.float16
X = mybir.AxisListType.X
ADD = mybir.AluOpType.add
MULT = mybir.AluOpType.mult
MAX = mybir.AluOpType.max
EXP = mybir.ActivationFunctionType.Exp
LOG = mybir.ActivationFunctionType.Ln

# problem constants (hardcoded per contract)
B = 8          # episodes = cores
Q = 75         # queries
P = 5          # ways (1-shot -> 1 proto per way)
C = 640        # channels
HW = 49        # spatial
QM = Q * HW    # 3675
PN = P * HW    # 245
PNP = 256      # padded moving dim for full-rate fp32r matmul
NT = 25        # partition-groups per way (75 q / 3)
NPART = 125    # 5 ways * 25
J = 3          # pairs per partition
E = HW * HW    # 2401
F = J * E      # 7203
CCH = 128      # contraction chunk
NCC = C // CCH # 5
TEMP = 12.5
EPS = 0.05
ITERS = 5
RSQC = 1.0 / np.sqrt(float(C))
WSCALE = 1.0 / HW
MARG_EPS = float(np.float32(1e-3) + np.float32(1e-5))

QMCH = [(k * 128, min(128, QM - k * 128)) for k in range((QM + 127) // 128)]  # 29
RCH = [(k * 512, min(512, QM - k * 512)) for k in range((QM + 511) // 512)]   # 8


def emit(tc, qry, sup, oh, ce_out, lg_out, gb, wb, qd, w1d, pd, zr):
    nc = tc.nc
    with ExitStack() as ctx:
        rows = ctx.enter_context(tc.tile_pool(name="rows", bufs=1))
        ev = ctx.enter_context(tc.tile_pool(name="ev", bufs=4))
        small = ctx.enter_context(tc.tile_pool(name="small", bufs=1))

        # ---------------- phase A: load inputs ----------------
        with tc.tile_pool(name="big", bufs=1) as big:
            QT = []
            ST = []
            QB = []
            SB = []
            qv = qry.rearrange("q c m -> c q m")
            sv = sup.rearrange("p c n -> c p n")
            for ci in range(NCC):
                # alternate the two HWDGE rings (SP / ACT) for load bandwidth
                dma_eng = nc.sync if ci % 2 == 0 else nc.scalar
                t = big.tile([128, QM], F32, tag=f"qt{ci}")
                dma_eng.dma_start(
                    t[:].rearrange("x (q m) -> x q m", q=Q),
                    qv[ci * CCH:(ci + 1) * CCH],
                )
                QT.append(t)
                s = big.tile([128, PNP], F32, tag=f"st{ci}")
                # zero only the pad columns: keeps the data region single-producer
                # (TensorReduce supports a single sync-wait slot in codegen)
                nc.vector.memset(s[:, PN:], 0.0)
                dma_eng.dma_start(
                    s[:, :PN].rearrange("x (p n) -> x p n", p=P),
                    sv[ci * CCH:(ci + 1) * CCH],
                )
                ST.append(s)
                # bf16 shadows for the PE (G + stats matmuls run in bf16)
                tb = big.tile([128, QM], BF16, tag=f"qb{ci}")
                nc.scalar.copy(tb[:], t[:])
                QB.append(tb)
                sb = big.tile([128, PNP], BF16, tag=f"sb{ci}")
                nc.scalar.copy(sb[:], s[:])
                SB.append(sb)

            # ---------------- phase B: stats + weight rows (PE/ACT/DVE) -------
            # gap sums (over spatial) for weight matmuls
            QG = []
            SG = []
            for ci in range(NCC):
                g = small.tile([128, Q], F32, tag=f"qg{ci}")
                nc.vector.tensor_reduce(
                    g[:], QT[ci][:].rearrange("x (q m) -> x q m", q=Q), axis=X, op=ADD
                )
                QG.append(g)
                h = small.tile([128, P], F32, tag=f"sg{ci}")
                nc.vector.tensor_reduce(
                    h[:], ST[ci][:, :PN].rearrange("x (p n) -> x p n", p=P),
                    axis=X, op=ADD,
                )
                SG.append(h)

            augq = rows.tile([1, QM], F32, tag="augq")
            augqb = rows.tile([1, QM], BF16, tag="augqb")
            ssqq = rows.tile([1, QM], F32, tag="ssqq")
            w1r = rows.tile([P, QM], F32, tag="w1r")
            augp = rows.tile([1, PNP], F32, tag="augp")
            augpb = rows.tile([1, PNP], BF16, tag="augpb")
            ssqp = rows.tile([1, PNP], F32, tag="ssqp")
            w2r = rows.tile([Q, PNP], F32, tag="w2r")

            onesb = small.tile([128, 1], BF16, tag="onesb")
            nc.vector.memset(onesb[:], 1.0)

            # col-sum rows: cmu_q (-> aug rows, bf16 matmul) ; w1 (fp32 matmul)
            with tc.tile_pool(name="ps1", bufs=2, space="PSUM") as ps1:
                for off, wd in RCH:
                    pc = ps1.tile([1, 512], F32, tag="prow")
                    pw = ps1.tile([P, 512], F32, tag="prow_w")
                    for ci in range(NCC):
                        nc.tensor.matmul(pc[:, :wd], onesb[:],
                                         QB[ci][:, off:off + wd],
                                         start=(ci == 0), stop=(ci == NCC - 1))
                        nc.tensor.matmul(pw[:, :wd], SG[ci][:],
                                         QT[ci][:, off:off + wd],
                                         start=(ci == 0), stop=(ci == NCC - 1))
                    nc.scalar.mul(augq[:, off:off + wd], pc[:, :wd], -RSQC)
                    nc.scalar.mul(augqb[:, off:off + wd], pc[:, :wd], -RSQC)
                    nc.scalar.mul(w1r[:, off:off + wd], pw[:, :wd], WSCALE)

            # ssq_q rows (bf16 squares via ACT, one reused tile)
            with tc.tile_pool(name="ps2", bufs=1, space="PSUM") as ps2:
                qsqb = big.tile([128, QM], BF16, tag="qsqb")
                pss = [
                    ps2.tile([1, 512], F32, tag=f"pss{k}", name=f"pss{k}")
                    for k in range(len(RCH))
                ]
                for ci in range(NCC):
                    nc.scalar.square(qsqb[:], QT[ci][:])
                    for k, (off, wd) in enumerate(RCH):
                        nc.tensor.matmul(pss[k][:, :wd], onesb[:],
                                         qsqb[:, off:off + wd],
                                         start=(ci == 0), stop=(ci == NCC - 1))
                for k, (off, wd) in enumerate(RCH):
                    nc.scalar.copy(ssqq[:, off:off + wd], pss[k][:, :wd])

            # sup-side rows: cmu_p, ssq_p (bf16), w2 (fp32)
            with tc.tile_pool(name="ps3", bufs=1, space="PSUM") as ps3:
                pcp = ps3.tile([1, PNP], F32, tag="pcp")
                psp = ps3.tile([1, PNP], F32, tag="psp")
                pw2 = ps3.tile([Q, PNP], F32, tag="pw2")
                ssb5 = ev.tile([128, PNP], BF16, tag="ssb5")
                for ci in range(NCC):
                    nc.tensor.matmul(pcp[:], onesb[:], SB[ci][:],
                                     start=(ci == 0), stop=(ci == NCC - 1))
                    nc.scalar.square(ssb5[:], ST[ci][:])
                    nc.tensor.matmul(psp[:], onesb[:], ssb5[:],
                                     start=(ci == 0), stop=(ci == NCC - 1))
                    nc.tensor.matmul(pw2[:], QG[ci][:], ST[ci][:],
                                     start=(ci == 0), stop=(ci == NCC - 1))
                nc.scalar.mul(augp[:], pcp[:], RSQC)
                nc.scalar.mul(augpb[:], pcp[:], RSQC)
                nc.scalar.copy(ssqp[:], psp[:])
                nc.scalar.mul(w2r[:], pw2[:], WSCALE)
                # bounce w2 through DRAM for the pair-major relayout
                nc.sync.dma_start(wb, w2r[:, :PN])

            # ---------------- phase C: G-hat matmuls (bf16) -> DRAM bounce ---
            with tc.tile_pool(name="ps4", bufs=4, space="PSUM") as ps4:
                for off, wd in QMCH:
                    pg = ps4.tile([128, PNP], F32, tag="pg")
                    for ci in range(NCC):
                        nc.tensor.matmul(
                            pg[:wd], QB[ci][:, off:off + wd], SB[ci][:],
                            start=(ci == 0), stop=False,
                        )
                    nc.tensor.matmul(
                        pg[:wd], augqb[:, off:off + wd], augpb[:],
                        start=False, stop=True,
                    )
                    ge = ev.tile([128, PN], F16, tag="ge")
                    nc.scalar.copy(ge[:wd], pg[:wd, :PN])
                    nc.scalar.dma_start(gb[off:off + wd, :], ge[:wd])

        # ---------------- phase D: pair-major relayouts ------------------
        pair = ctx.enter_context(tc.tile_pool(name="pair", bufs=1))
        # Single-DMA gathers: each pair-layout tile has exactly one producer so
        # consumers never exceed the per-instruction sync-wait limit. The small
        # stat rows bounce through DRAM (qd) for the same reason.
        GP = pair.tile([NPART, F], F16, tag="gp")
        nc.sync.dma_start(
            GP[:].rearrange("x (j m n) -> x j m n", j=J, m=HW),
            gb.rearrange("(t j m) (p n) -> p t j m n", t=NT, j=J, p=P),
        )

        AQP = small.tile([NPART, HW * J], F32, tag="aqp")
        SQP = small.tile([NPART, HW * J], F32, tag="sqp")
        W1P = small.tile([NPART, HW * J], F32, tag="w1p")
        APP = small.tile([NPART, HW * J], F32, tag="app")
        SPP = small.tile([NPART, HW * J], F32, tag="spp")
        W2P = small.tile([NPART, HW * J], F32, tag="w2p")

        # dump the stat rows to DRAM (single producers for the gathers below)
        nc.sync.dma_start(qd[0], augq[:])
        nc.sync.dma_start(qd[1], ssqq[:])
        nc.scalar.dma_start(w1d, w1r[:])
        nc.sync.dma_start(pd[0], augp[:, :PN])
        nc.sync.dma_start(pd[1], ssqp[:, :PN])

        def rep_q(x):  # [QM] -> [p(step0), tjm]  replicated over p
            return x.broadcast_to((QM, P)).rearrange("f p -> p f")

        nc.sync.dma_start(AQP[:], rep_q(qd[0]))
        nc.sync.dma_start(SQP[:], rep_q(qd[1]))
        nc.scalar.dma_start(W1P[:], w1d)

        def rep_p(x):  # [PN] -> [p, tj(step0), n]  broadcast over (t, j)
            return x.rearrange("(p n) -> p n", p=P) \
                    .broadcast_to((P, HW, NT * J)).rearrange("p n t -> p t n")

        nc.sync.dma_start(APP[:].rearrange("x (j n) -> x j n", j=J), rep_p(pd[0]))
        nc.sync.dma_start(SPP[:].rearrange("x (j n) -> x j n", j=J), rep_p(pd[1]))
        nc.scalar.dma_start(
            W2P[:].rearrange("x (j n) -> x j n", j=J),
            wb.rearrange("q (p n) -> p q n", p=P),
        )

        # ---------------- phase E: r-vectors, S, K, SK, marginals --------
        def rsqrt_nr(dstag, aug_t, ssq_t):
            t1 = small.tile([NPART, HW * J], F32, tag="sc1")
            nc.vector.tensor_mul(t1[:], aug_t[:], aug_t[:])
            nsq = small.tile([NPART, HW * J], F32, tag="sc2")
            nc.vector.tensor_sub(nsq[:], ssq_t[:], t1[:])
            nc.vector.tensor_scalar_max(nsq[:], nsq[:], 1e-16)
            sq = small.tile([NPART, HW * J], F32, tag="sc3")
            nc.scalar.sqrt(sq[:], nsq[:])
            y0 = small.tile([NPART, HW * J], F32, tag="sc4")
            nc.vector.reciprocal(y0[:], sq[:])
            # NR: y1 = y0 * (1.5 - 0.5 * nsq * y0^2)
            nc.vector.tensor_mul(t1[:], y0[:], y0[:])
            nc.vector.tensor_mul(t1[:], t1[:], nsq[:])
            nc.vector.tensor_scalar(t1[:], t1[:], -0.5, 1.5, op0=MULT, op1=ADD)
            out = small.tile([NPART, HW * J], F32, tag=dstag)
            nc.vector.tensor_mul(out[:], y0[:], t1[:])
            return out

        RQ = rsqrt_nr("rq", AQP, SQP)
        RP = rsqrt_nr("rp", APP, SPP)

        # S = G * rq (bcast n) * rp (bcast m);  G pair tile is m-major [j][m][n]
        TF = pair.tile([NPART, F], F32, tag="tf")
        nc.vector.tensor_mul(
            TF[:].rearrange("x (j m n) -> x j m n", j=J, m=HW),
            GP[:].rearrange("x (j m n) -> x j m n", j=J, m=HW),
            RQ[:].rearrange("x (j m) -> x j m", j=J).broadcast_to((NPART, J, HW, HW)),
        )
        SP = pair.tile([NPART, F], F32, tag="sp")  # n-major [j][n][m]
        nc.vector.tensor_mul(
            SP[:].rearrange("x (j n m) -> x j n m", j=J, n=HW),
            TF[:].rearrange("x (j m n) -> x j n m", j=J, m=HW),
            RP[:].rearrange("x (j n) -> x j n", j=J).broadcast_to((NPART, J, HW, HW)),
        )
        KK = pair.tile([NPART, F], F32, tag="kk")  # n-major
        bm20 = small.tile([NPART, 1], F32, tag="bm20")
        nc.vector.memset(bm20[:], -1.0 / EPS)
        nc.scalar.activation(KK[:], SP[:], EXP, bias=bm20[:], scale=1.0 / EPS)

        # marginals a (j,m-order), b (j,n-order)
        def marginal(dstag, wsrc):
            wa = small.tile([NPART, HW * J], F32, tag="sc1")
            nc.vector.tensor_scalar(wa[:], wsrc[:], 0.0, MARG_EPS, op0=MAX, op1=ADD)
            sa = small.tile([NPART, J], F32, tag="sc5")
            nc.vector.tensor_reduce(
                sa[:], wa[:].rearrange("x (j m) -> x j m", j=J), axis=X, op=ADD)
            ra = small.tile([NPART, J], F32, tag="sc6")
            nc.vector.reciprocal(ra[:], sa[:])
            out = small.tile([NPART, HW * J], F32, tag=dstag)
            nc.vector.tensor_mul(
                out[:].rearrange("x (j m) -> x j m", j=J),
                wa[:].rearrange("x (j m) -> x j m", j=J),
                ra[:].broadcast_to((NPART, J, HW)),
            )
            return out

        AT = marginal("aa", W1P)
        BT = marginal("bb", W2P)

        # ---------------- phase F: Sinkhorn scaling iterations -----------
        U = small.tile([NPART, HW * J], F32, tag="uu")
        V = small.tile([NPART, HW * J], F32, tag="vv")
        kk_jnm = KK[:].rearrange("x (j n m) -> x j n m", j=J, n=HW)   # natural
        kk_jmn = KK[:].rearrange("x (j n m) -> x j m n", j=J, n=HW)   # transposed view
        rscr = small.tile([NPART, HW * J], F32, tag="rscr")
        for it in range(ITERS):
            su = small.tile([NPART, HW * J], F32, tag="sc1")
            if it == 0:
                # v == 1 -> t = K; reduce K directly
                nc.vector.tensor_reduce(su[:], kk_jmn, axis=X, op=ADD)
            else:
                # t = K * v  (iterate j,n,m; v bcast over m)
                nc.vector.tensor_mul(
                    TF[:].rearrange("x (j n m) -> x j n m", j=J, n=HW),
                    kk_jnm,
                    V[:].rearrange("x (j n) -> x j n", j=J)
                        .broadcast_to((NPART, J, HW, HW)),
                )
                nc.vector.tensor_reduce(
                    su[:], TF[:].rearrange("x (j n m) -> x j m n", j=J, n=HW),
                    axis=X, op=ADD)
            ru = small.tile([NPART, HW * J], F32, tag="sc2")
            nc.vector.reciprocal_approx_fast(ru[:], su[:])
            nc.vector.tensor_mul(U[:], AT[:], ru[:])
            # t2 = K * u (iterate j,m,n; u bcast over n)
            nc.vector.tensor_mul(
                TF[:].rearrange("x (j m n) -> x j m n", j=J, m=HW),
                kk_jmn,
                U[:].rearrange("x (j m) -> x j m", j=J).broadcast_to((NPART, J, HW, HW)),
            )
            sv = small.tile([NPART, HW * J], F32, tag="sc3")
            nc.vector.tensor_reduce(
                sv[:], TF[:].rearrange("x (j m n) -> x j n m", j=J, m=HW),
                axis=X, op=ADD)
            rv = small.tile([NPART, HW * J], F32, tag="sc4")
            if it == ITERS - 1:
                nc.vector.reciprocal_approx_accurate(rv[:], sv[:], rscr[:])
            else:
                nc.vector.reciprocal_approx_fast(rv[:], sv[:])
            nc.vector.tensor_mul(V[:], BT[:], rv[:])

        # ---------------- phase G: logits + CE ---------------------------
        # z = sum_n v_n * sum_m S[j,m,n] * t2[j,m,n], with t2 = K*u from the
        # final iteration still in TF (m-major).
        T3 = pair.tile([NPART, F], F32, tag="gp")  # reuse GP slot
        nc.vector.tensor_mul(
            T3[:].rearrange("x (j m n) -> x j m n", j=J, m=HW),
            TF[:].rearrange("x (j m n) -> x j m n", j=J, m=HW),
            SP[:].rearrange("x (j n m) -> x j m n", j=J, n=HW),
        )
        sm = small.tile([NPART, HW * J], F32, tag="sc1")
        nc.vector.tensor_reduce(
            sm[:], T3[:].rearrange("x (j m n) -> x j n m", j=J, m=HW),
            axis=X, op=ADD)
        t4 = small.tile([NPART, HW * J], F32, tag="sc2")
        nc.vector.tensor_mul(t4[:], sm[:], V[:])
        Z = small.tile([NPART, J], F32, tag="zz")
        nc.vector.tensor_reduce(
            Z[:], t4[:].rearrange("x (j n) -> x j n", j=J), axis=X, op=ADD)

        # Z [(p t), j] -> DRAM -> L [q, p]  (single producer for the CE ops)
        nc.sync.dma_start(zr, Z[:])
        L = small.tile([Q, P], F32, tag="ll")
        nc.sync.dma_start(
            L[:],
            zr.rearrange("(p t) j -> (t j) p", p=P),
        )

        OH = small.tile([Q, P], F32, tag="oh")
        nc.sync.dma_start(OH[:], oh)

        mx = small.tile([Q, 1], F32, tag="mx")
        nc.vector.tensor_reduce(mx[:], L[:], axis=X, op=MAX)
        nmx = small.tile([Q, 1], F32, tag="nmx")
        nc.vector.tensor_scalar_mul(nmx[:], mx[:], -TEMP)
        ee = small.tile([Q, P], F32, tag="ee")
        nc.scalar.activation(ee[:], L[:], EXP, bias=nmx[:], scale=TEMP)
        se = small.tile([Q, 1], F32, tag="se")
        nc.vector.tensor_reduce(se[:], ee[:], axis=X, op=ADD)
        lg = small.tile([Q, 1], F32, tag="lgs")
        zb = small.tile([Q, 1], F32, tag="zb")
        nc.vector.memset(zb[:], 0.0)
        nc.scalar.activation(lg[:], se[:], LOG, bias=zb[:])
        zl5 = small.tile([Q, P], F32, tag="zl5")
        nc.vector.tensor_mul(zl5[:], L[:], OH[:])
        zl = small.tile([Q, 1], F32, tag="zl")
        nc.vector.tensor_reduce(zl[:], zl5[:], axis=X, op=ADD)
        d1 = small.tile([Q, 1], F32, tag="d1")
        nc.vector.tensor_sub(d1[:], mx[:], zl[:])
        ceo = small.tile([Q, 1], F32, tag="ceo")
        nc.vector.scalar_tensor_tensor(ceo[:], d1[:], TEMP, lg[:], op0=MULT, op1=ADD)

        nc.sync.dma_start(ce_out, ceo[:])
        nc.sync.dma_start(lg_out, L[:])


def build_program():
    nc = bacc.Bacc("TRN2", target_bir_lowering=False, debug=False)
    qry = nc.dram_tensor("qry", [Q, C, HW], F32, kind="ExternalInput").ap()
    sup = nc.dram_tensor("sup", [P, C, HW], F32, kind="ExternalInput").ap()
    oh = nc.dram_tensor("oh", [Q, P], F32, kind="ExternalInput").ap()
    ce = nc.dram_tensor("ce", [Q, 1], F32, kind="ExternalOutput").ap()
    lgt = nc.dram_tensor("lgt", [Q, P], F32, kind="ExternalOutput").ap()
    gb = nc.dram_tensor("gb", [QM, PN], F16).ap()
    wb = nc.dram_tensor("wb", [Q, PN], F32).ap()
    qd = nc.dram_tensor("qd", [2, QM], F32).ap()
    w1d = nc.dram_tensor("w1d", [P, QM], F32).ap()
    pd = nc.dram_tensor("pd", [2, PN], F32).ap()
    zr = nc.dram_tensor("zr", [NPART, J], F32).ap()
    with tile.TileContext(nc) as tc:
        emit(tc, qry, sup, oh, ce, lgt, gb, wb, qd, w1d, pd, zr)
    nc.compile()
    return nc


def make_in_maps(support_xf, query_xf, query_y):
    support_xf = np.ascontiguousarray(np.asarray(support_xf, dtype=np.float32))
    query_xf = np.ascontiguousarray(np.asarray(query_xf, dtype=np.float32))
    query_y = np.asarray(query_y)
    in_maps = []
    for i in range(B):
        ohm = np.zeros((Q, P), np.float32)
        ohm[np.arange(Q), query_y[i].astype(np.int64)] = 1.0
        in_maps.append({
            "qry": query_xf[i].reshape(Q, C, HW),
            "sup": support_xf[i].reshape(P, C, HW),
            "oh": ohm,
        })
    return in_maps


def kernel(support_xf, query_xf, support_y, query_y, n_way=5, k_shot=1, **_):
    nc = build_program()
    in_maps = make_in_maps(support_xf, query_xf, query_y)
    res = run_bass_kernel_spmd(nc, in_maps, list(range(B)))
    ce = np.concatenate([res.results[i]["ce"].reshape(-1) for i in range(B)])
    return np.float32(ce.mean())



# revision 16
# speedup vs baseline: 5.9938x; 5.9938x over previous
"""DeepEMD episode loss kernel for Trainium2 (8 NeuronCores, data-parallel over episodes).

Per core = one episode (75 queries x 5 protos, 640ch, 7x7 spatial):
  - support side is centered+scaled on-chip (sup-cmu)*rp before the cosine
    matmul, so G = qry_raw^T @ supn_scaled = S/rq directly (the centering
    rank-1 correction vanishes because sum_c supn = 0).
  - G computed in bf16 on the PE; written to DRAM already in pair-major
    order (piecewise t-aligned scatter, ~5 contiguous runs per piece), then
    read back as one contiguous [125, 7203] tile -- no 18k-descriptor gather.
  - Sinkhorn in scaling form, 2 iterations (matches the 100-iter log-domain
    reference to ~4e-5), big elementwise ops in bf16 with packed last dims
    (DVE 2x mode); K kept in both m-major and n-major copies so every mul
    and reduce is unit-stride.
  - logits z = T * u.(SK v); per-query CE returned, mean taken on host.
"""

import numpy as np
from contextlib import ExitStack

import concourse.bass as bass
import concourse.bacc as bacc
import concourse.tile as tile
from concourse import mybir
from concourse.bass_utils import run_bass_kernel_spmd

F32 = mybir.dt.float32
BF16 = mybir.dt.bfloat16
X = mybir.AxisListType.X
ADD = mybir.AluOpType.add
MULT = mybir.AluOpType.mult
MAX = mybir.AluOpType.max
EXP = mybir.ActivationFunctionType.Exp
LOG = mybir.ActivationFunctionType.Ln

# problem constants (hardcoded per contract)
B = 8          # episodes = cores
Q = 75         # queries
P = 5          # ways (1-shot -> 1 proto per way)
C = 640        # channels
HW = 49        # spatial
QM = Q * HW    # 3675
PN = P * HW    # 245
NT = 25        # t-groups (3 queries each)
NPART = 125    # (t, p) pairs
J = 3          # queries per t-group
E = HW * HW    # 2401
F = J * E      # 7203
CCH = 128      # contraction chunk
NCC = C // CCH # 5
TEMP = 12.5
EPS = 0.05
ITERS = 2
DEBUG = False
RSQC = 1.0 / np.sqrt(float(C))
GSCALE = 1.0 / HW
MARG_EPS = float(np.float32(1e-3) + np.float32(1e-5))

QMCH = [(k * 128, min(128, QM - k * 128)) for k in range((QM + 127) // 128)]  # 29
RCH = [(k * 512, min(512, QM - k * 512)) for k in range((QM + 511) // 512)]   # 8


def t_spans(off, wd):
    """Split global row range [off, off+wd) at t-group boundaries (147 rows).
    Yields (t, r0, r1, fo): local rows [r0, r1), pair free offset fo=j*49+m."""
    g0, g1 = off, off + wd
    t = g0 // (J * HW)
    while t * J * HW < g1:
        s0 = max(g0, t * J * HW)
        s1 = min(g1, (t + 1) * J * HW)
        yield t, s0 - g0, s1 - g0, s0 - t * J * HW
        t += 1


def emit(tc, qry, sup, oh, ce_out, cpd, rpd, gb2, qd, w1d, w2d, zr):
    nc = tc.nc
    with ExitStack() as ctx:
        pers = ctx.enter_context(tc.tile_pool(name="pers", bufs=1))
        pair = ctx.enter_context(tc.tile_pool(name="pair", bufs=1))
        small = ctx.enter_context(tc.tile_pool(name="small", bufs=1))

        onesb = small.tile([128, 1], BF16, name="onesb")
        nc.vector.memset(onesb[:], 1.0)
        OH = small.tile([Q, P], F32, name="oh_t")
        nc.sync.dma_start(OH[:], oh)

        QB = []     # bf16 query shadows [128, QM] x5
        SBn = []    # centered+scaled sup bf16 [128, PN] x5
        SBb = []    # raw sup bf16 (for w2/colsum) x5
        AQP = small.tile([NPART, HW * J], F32, name="aqp")
        SQP = small.tile([NPART, HW * J], F32, name="sqp")
        W1P = small.tile([NPART, HW * J], F32, name="w1p")
        W2P = small.tile([NPART, HW * J], F32, name="w2p")
        GP = pair.tile([NPART, F], BF16, name="gp")

        lctx = ExitStack()
        loadp = lctx.enter_context(tc.tile_pool(name="loadp", bufs=1))
        qtp = lctx.enter_context(tc.tile_pool(name="qtp", bufs=2))
        qsqp = lctx.enter_context(tc.tile_pool(name="qsqp", bufs=1))
        if True:
            sv = sup.rearrange("p c n -> c p n")
            qv = qry.rearrange("q c m -> c q m")

            # ---- support side: load, stats, center+scale ----
            ST = []
            with tc.tile_pool(name="psS", bufs=1, space="PSUM") as psS:
                pcp = psS.tile([1, PN], F32, name="pcp")
                psp = psS.tile([1, PN], F32, name="psp")
                for ci in range(NCC):
                    st = loadp.tile([128, PN], F32, name=f"st{ci}")
                    nc.sync.dma_start(
                        st[:].rearrange("x (p n) -> x p n", p=P),
                        sv[ci * CCH:(ci + 1) * CCH])
                    ST.append(st)
                    sb = pers.tile([128, PN], BF16, name=f"sbb{ci}")
                    nc.scalar.copy(sb[:], st[:])
                    SBb.append(sb)
                ssb = loadp.tile([128, PN], BF16, name="ssb")
                for ci in range(NCC):
                    nc.tensor.matmul(pcp[:], onesb[:], SBb[ci][:],
                                     start=(ci == 0), stop=(ci == NCC - 1))
                for ci in range(NCC):
                    nc.scalar.square(ssb[:], ST[ci][:])
                    nc.tensor.matmul(psp[:], onesb[:], ssb[:],
                                     start=(ci == 0), stop=(ci == NCC - 1))
                # rows: cp = pcp/C ; nsq = ssq - C*cp^2 ; rp = rsqrt(nsq) (+1 NR)
                cp_row = pers.tile([1, PN], F32, name="cp_row")
                nc.scalar.mul(cp_row[:], pcp[:], 1.0 / C)
                ssq_row = small.tile([1, PN], F32, name="ssq_row")
                nc.scalar.copy(ssq_row[:], psp[:])
            t1r = small.tile([1, PN], F32, name="t1r")
            nc.vector.tensor_mul(t1r[:], cp_row[:], cp_row[:])
            nsqr = small.tile([1, PN], F32, name="nsqr")
            nc.vector.scalar_tensor_tensor(nsqr[:], t1r[:], -float(C), ssq_row[:],
                                           op0=MULT, op1=ADD)
            nc.vector.tensor_scalar_max(nsqr[:], nsqr[:], 1e-16)
            sqr = small.tile([1, PN], F32, name="sqr")
            nc.scalar.sqrt(sqr[:], nsqr[:])
            y0r = small.tile([1, PN], F32, name="y0r")
            nc.vector.reciprocal(y0r[:], sqr[:])
            nc.vector.tensor_mul(t1r[:], y0r[:], y0r[:])
            nc.vector.tensor_mul(t1r[:], t1r[:], nsqr[:])
            nc.vector.tensor_scalar(t1r[:], t1r[:], -0.5, 1.5, op0=MULT, op1=ADD)
            rp_row = pers.tile([1, PN], F32, name="rp_row")
            nc.vector.tensor_mul(rp_row[:], y0r[:], t1r[:])

            # broadcast cp/rp rows to all 128 partitions via DRAM bounce
            nc.sync.dma_start(cpd, cp_row[:])
            nc.sync.dma_start(rpd, rp_row[:])
            cpb = pers.tile([128, PN], F32, name="cpb")
            rpb = pers.tile([128, PN], F32, name="rpb")
            nc.sync.dma_start(cpb[:], cpd.broadcast_to((PN, 128)).rearrange("f x -> x f"))
            nc.sync.dma_start(rpb[:], rpd.broadcast_to((PN, 128)).rearrange("f x -> x f"))
            sgf = small.tile([128, P], F32, name="sgf")
            sctmp = loadp.tile([128, PN], F32, name="sctmp")
            for ci in range(NCC):
                nc.vector.tensor_sub(sctmp[:], ST[ci][:], cpb[:])
                sbn = pers.tile([128, PN], BF16, name=f"sbn{ci}")
                nc.vector.tensor_mul(sbn[:], sctmp[:], rpb[:])
                SBn.append(sbn)

            # stacked lhsT for [colsum; w1] matmuls: col0 = ones, cols1-5 = sup_gap/HW
            SGO = pers.tile([128, 1 + P], BF16, name="sgo")
            nc.vector.memset(SGO[:, 0:1], 1.0)
            nc.vector.tensor_reduce(
                sgf[:], ST[0][:].rearrange("x (p n) -> x p n", p=P), axis=X, op=ADD)
            nc.scalar.mul(SGO[:, 1:1 + P], sgf[:], GSCALE)

            # ---- query side: load + per-chunk stats (pipelined under loads) ----
            with tc.tile_pool(name="psQ", bufs=1, space="PSUM") as psQ:
                pw6 = psQ.tile([1 + P, 512], F32, name="pw6")
                # 8 ssq accumulators: rows {0,32} x 4 column slots of 512
                pss = psQ.tile([33, 4 * 512], F32, name="pss")
                pw2 = psQ.tile([Q, PN], F32, name="pw2")
                for ci in range(NCC):
                    qt = qtp.tile([128, QM], F32, name="qt")
                    dma_eng = nc.sync if ci % 2 == 0 else nc.scalar
                    dma_eng.dma_start(
                        qt[:].rearrange("x (q m) -> x q m", q=Q),
                        qv[ci * CCH:(ci + 1) * CCH])
                    qb = pers.tile([128, QM], BF16, name=f"qb{ci}")
                    nc.scalar.copy(qb[:], qt[:])
                    QB.append(qb)
                    qsq = qsqp.tile([128, QM], BF16, name="qsq")
                    nc.scalar.square(qsq[:], qb[:])
                    qg = small.tile([128, Q], F32, name=f"qg{ci}")
                    nc.vector.tensor_reduce(
                        qg[:], qb[:].rearrange("x (q m) -> x q m", q=Q), axis=X, op=ADD)
                    qgb = small.tile([128, Q], BF16, name=f"qgb{ci}")
                    nc.scalar.mul(qgb[:], qg[:], GSCALE)
                    for k, (off, wd) in enumerate(RCH):
                        nc.tensor.matmul(pw6[:, :wd], SGO[:], qb[:, off:off + wd],
                                         start=(ci == 0), stop=(ci == NCC - 1))
                    for k, (off, wd) in enumerate(RCH):
                        r0, c0 = 32 * (k // 4), 512 * (k % 4)
                        nc.tensor.matmul(pss[r0:r0 + 1, c0:c0 + wd], onesb[:],
                                         qsq[:, off:off + wd],
                                         start=(ci == 0), stop=(ci == NCC - 1))
                    nc.tensor.matmul(pw2[:], qgb[:], SBb[ci][:],
                                     start=(ci == 0), stop=(ci == NCC - 1))

                # ---- stat rows -> DRAM (load pools freed first) ----
                lctx.close()
                with tc.tile_pool(name="rows", bufs=1) as rows:
                    augq = rows.tile([1, QM], F32, name="augq")
                    ssqq = rows.tile([1, QM], F32, name="ssqq")
                    w16r = rows.tile([1 + P, QM], F32, name="w16r")
                    w2r = rows.tile([Q, PN], F32, name="w2r")
                    for k, (off, wd) in enumerate(RCH):
                        r0, c0 = 32 * (k // 4), 512 * (k % 4)
                        nc.scalar.copy(w16r[:, off:off + wd], pw6[:, :wd])
                        nc.scalar.mul(augq[:, off:off + wd], w16r[0:1, off:off + wd],
                                      RSQC)
                        nc.scalar.copy(ssqq[:, off:off + wd], pss[r0:r0 + 1, c0:c0 + wd])
                    nc.scalar.copy(w2r[:], pw2[:])
                    nc.sync.dma_start(qd[0], augq[:])
                    nc.sync.dma_start(qd[1], ssqq[:])
                    nc.scalar.dma_start(w1d, w16r[1:1 + P, :])
                    nc.scalar.dma_start(w2d, w2r[:])

                # ---- pair-space relayouts, (p,t) order (DRAM -> SBUF) ----
                def rep_q(x):  # [QM] -> [(p,t), f] replicated over p
                    return x.broadcast_to((QM, P)).rearrange("f p -> p f")

                nc.sync.dma_start(AQP[:], rep_q(qd[0]))
                nc.sync.dma_start(SQP[:], rep_q(qd[1]))
                nc.scalar.dma_start(W1P[:], w1d)
                nc.scalar.dma_start(
                    W2P[:].rearrange("x (j n) -> x j n", j=J),
                    w2d.rearrange("q (p n) -> p q n", p=P))

            # ---- G matmuls, scatter to DRAM in pair order, read back ----
            with tc.tile_pool(name="psG", bufs=4, space="PSUM") as psG, \
                 tc.tile_pool(name="gep", bufs=4) as gep:
                for gi, (off, wd) in enumerate(QMCH):
                    pg = psG.tile([128, PN], F32, name="pg")
                    for ci in range(NCC):
                        nc.tensor.matmul(pg[:wd], QB[ci][:, off:off + wd], SBn[ci][:],
                                         start=(ci == 0), stop=(ci == NCC - 1))
                    ge = gep.tile([128, PN], BF16, name="ge")
                    nc.scalar.copy(ge[:wd], pg[:wd])
                    for t, r0, r1, fo in t_spans(off, wd):
                        eng = nc.scalar if (gi % 2 == 0) else nc.sync
                        dst = gb2.rearrange("(p t) f -> t p f", t=NT)[t]
                        eng.dma_start(
                            dst[:, fo * HW:(fo + (r1 - r0)) * HW]
                                .rearrange("p (f n) -> f p n", n=HW),
                            ge[r0:r1, :].rearrange("f (p n) -> f p n", p=P),
                        )
            nc.sync.dma_start(GP[:], gb2)

        # ---- pair-space scalar prep: rq, marginals ----
        def rsqrt_nr(dsname, aug_t, ssq_t):
            t1 = small.tile([NPART, HW * J], F32, name="sc1")
            nc.vector.tensor_mul(t1[:], aug_t[:], aug_t[:])
            nsq = small.tile([NPART, HW * J], F32, name="sc2")
            nc.vector.tensor_sub(nsq[:], ssq_t[:], t1[:])
            nc.vector.tensor_scalar_max(nsq[:], nsq[:], 1e-16)
            sq = small.tile([NPART, HW * J], F32, name="sc3")
            nc.scalar.sqrt(sq[:], nsq[:])
            y0 = small.tile([NPART, HW * J], F32, name="sc4")
            nc.vector.reciprocal(y0[:], sq[:])
            nc.vector.tensor_mul(t1[:], y0[:], y0[:])
            nc.vector.tensor_mul(t1[:], t1[:], nsq[:])
            nc.vector.tensor_scalar(t1[:], t1[:], -0.5, 1.5, op0=MULT, op1=ADD)
            out = small.tile([NPART, HW * J], F32, name=dsname)
            nc.vector.tensor_mul(out[:], y0[:], t1[:])
            return out

        RQ = rsqrt_nr("rq", AQP, SQP)

        def marginal(dsname, wsrc):
            wa = small.tile([NPART, HW * J], F32, name="sc1")
            nc.vector.tensor_scalar(wa[:], wsrc[:], 0.0, MARG_EPS, op0=MAX, op1=ADD)
            sa = small.tile([NPART, J], F32, name="sc5")
            nc.vector.tensor_reduce(
                sa[:], wa[:].rearrange("x (j m) -> x j m", j=J), axis=X, op=ADD)
            ra = small.tile([NPART, J], F32, name="sc6")
            nc.vector.reciprocal(ra[:], sa[:])
            out = small.tile([NPART, HW * J], F32, name=dsname)
            nc.vector.tensor_mul(
                out[:].rearrange("x (j m) -> x j m", j=J),
                wa[:].rearrange("x (j m) -> x j m", j=J),
                ra[:].broadcast_to((NPART, J, HW)),
            )
            return out

        AT = marginal("aa", W1P)   # row marginal a [x, (j,m)]
        BT = marginal("bb", W2P)   # col marginal b [x, (j,n)]

        # ---- S (both layouts), K, SK ----
        Sn = pair.tile([NPART, F], BF16, name="sn")
        nc.vector.tensor_mul(
            Sn[:].rearrange("x (j n m) -> x j n m", j=J, n=HW),
            GP[:].rearrange("x (j m n) -> x j n m", j=J, m=HW),
            RQ[:].rearrange("x (j o m) -> x j o m", j=J, o=1)
                .broadcast_to((NPART, J, HW, HW)),
        )
        Sm = GP  # in-place scale: GP dead after this (Sn already built)
        nc.vector.tensor_mul(
            Sm[:].rearrange("x (j m n) -> x j m n", j=J, m=HW),
            GP[:].rearrange("x (j m n) -> x j m n", j=J, m=HW),
            RQ[:].rearrange("x (j m) -> x j m", j=J)
                .broadcast_to((NPART, J, HW, HW)),
        )
        bm20 = small.tile([NPART, 1], F32, name="bm20")
        nc.vector.memset(bm20[:], -1.0 / EPS)
        Kn = pair.tile([NPART, F], BF16, name="kn")
        nc.scalar.activation(Kn[:], Sn[:], EXP, bias=bm20[:], scale=1.0 / EPS)
        Km = pair.tile([NPART, F], BF16, name="km")
        nc.scalar.activation(Km[:], Sm[:], EXP, bias=bm20[:], scale=1.0 / EPS)
        SK = Sn  # in-place: Sn dead once Kn is computed
        nc.vector.tensor_mul(SK[:], Sn[:], Kn[:])

        # ---- Sinkhorn scaling iterations (packed bf16 muls, f32 reduces) ----
        TT = pair.tile([NPART, F], BF16, name="tt")
        km_v = Km[:].rearrange("x (j m n) -> x j m n", j=J, m=HW)
        kn_v = Kn[:].rearrange("x (j n m) -> x j n m", j=J, n=HW)
        tt_m = TT[:].rearrange("x (j m n) -> x j m n", j=J, m=HW)
        tt_n = TT[:].rearrange("x (j n m) -> x j n m", j=J, n=HW)
        su = small.tile([NPART, HW * J], F32, name="su")
        sv_t = small.tile([NPART, HW * J], F32, name="sv_t")
        ru = small.tile([NPART, HW * J], F32, name="ru")
        rv = small.tile([NPART, HW * J], F32, name="rv")
        U = small.tile([NPART, HW * J], BF16, name="uu")
        V = small.tile([NPART, HW * J], BF16, name="vv")
        for it in range(ITERS):
            if it == 0:
                nc.vector.tensor_reduce(su[:], km_v, axis=X, op=ADD)
            else:
                nc.vector.tensor_mul(
                    tt_m, km_v,
                    V[:].rearrange("x (j o n) -> x j o n", j=J, o=1)
                        .broadcast_to((NPART, J, HW, HW)),
                )
                nc.vector.tensor_reduce(su[:], tt_m, axis=X, op=ADD)
            nc.vector.reciprocal_approx_fast(ru[:], su[:])
            nc.vector.tensor_mul(U[:], AT[:], ru[:])
            nc.vector.tensor_mul(
                tt_n, kn_v,
                U[:].rearrange("x (j o m) -> x j o m", j=J, o=1)
                    .broadcast_to((NPART, J, HW, HW)),
            )
            nc.vector.tensor_reduce(sv_t[:], tt_n, axis=X, op=ADD)
            nc.vector.reciprocal_approx_fast(rv[:], sv_t[:])
            nc.vector.tensor_mul(V[:], BT[:], rv[:])

        # ---- logits z = sum_n v * sum_m (S K) u ----
        nc.vector.tensor_mul(
            tt_n, SK[:].rearrange("x (j n m) -> x j n m", j=J, n=HW),
            U[:].rearrange("x (j o m) -> x j o m", j=J, o=1)
                .broadcast_to((NPART, J, HW, HW)),
        )
        sm_r = small.tile([NPART, HW * J], F32, name="sm_r")
        nc.vector.tensor_reduce(sm_r[:], tt_n, axis=X, op=ADD)
        t4 = small.tile([NPART, HW * J], F32, name="t4")
        nc.vector.tensor_mul(t4[:], sm_r[:], V[:])
        Z = small.tile([NPART, J], F32, name="zz")
        nc.vector.tensor_reduce(
            Z[:], t4[:].rearrange("x (j n) -> x j n", j=J), axis=X, op=ADD)

        # ---- Z[(t,p), j] -> L[q, p] via DRAM, then CE ----
        nc.sync.dma_start(zr, Z[:])
        L = small.tile([Q, P], F32, name="ll")
        nc.sync.dma_start(L[:], zr.rearrange("(p t) j -> (t j) p", p=P))
        mx = small.tile([Q, 1], F32, name="mx")
        nc.vector.tensor_reduce(mx[:], L[:], axis=X, op=MAX)
        nmx = small.tile([Q, 1], F32, name="nmx")
        nc.vector.tensor_scalar_mul(nmx[:], mx[:], -TEMP)
        ee = small.tile([Q, P], F32, name="ee")
        nc.scalar.activation(ee[:], L[:], EXP, bias=nmx[:], scale=TEMP)
        se = small.tile([Q, 1], F32, name="se")
        nc.vector.tensor_reduce(se[:], ee[:], axis=X, op=ADD)
        lg = small.tile([Q, 1], F32, name="lgs")
        zb = small.tile([Q, 1], F32, name="zb")
        nc.vector.memset(zb[:], 0.0)
        nc.scalar.activation(lg[:], se[:], LOG, bias=zb[:])
        zl5 = small.tile([Q, P], F32, name="zl5")
        nc.vector.tensor_mul(zl5[:], L[:], OH[:])
        zl = small.tile([Q, 1], F32, name="zl")
        nc.vector.tensor_reduce(zl[:], zl5[:], axis=X, op=ADD)
        d1 = small.tile([Q, 1], F32, name="d1")
        nc.vector.tensor_sub(d1[:], mx[:], zl[:])
        ceo = small.tile([Q, 1], F32, name="ceo")
        nc.vector.scalar_tensor_tensor(ceo[:], d1[:], TEMP, lg[:], op0=MULT, op1=ADD)
        nc.sync.dma_start(ce_out, ceo[:])
        if DEBUG:
            dbg = emit.dbg
            nc.sync.dma_start(dbg["gp"], Sm[:])     # after in-place: Sm (=GP tile)
            nc.sync.dma_start(dbg["sn"], SK[:])     # after in-place: SK (=Sn tile)
            nc.sync.dma_start(dbg["kn"], Kn[:])
            nc.sync.dma_start(dbg["km"], Km[:])
            nc.sync.dma_start(dbg["rq"], RQ[:])
            nc.sync.dma_start(dbg["at"], AT[:])
            nc.sync.dma_start(dbg["bt"], BT[:])
            nc.sync.dma_start(dbg["su"], su[:])
            nc.sync.dma_start(dbg["sv"], sv_t[:])
            nc.sync.dma_start(dbg["uu"], U[:])
            nc.sync.dma_start(dbg["vv"], V[:])
            nc.sync.dma_start(dbg["zz"], Z[:])
            nc.sync.dma_start(dbg["ll"], L[:])
            nc.sync.dma_start(dbg["aqp"], AQP[:])
            nc.sync.dma_start(dbg["sqp"], SQP[:])
            nc.sync.dma_start(dbg["qd1"], qd[1])


def build_program():
    nc = bacc.Bacc("TRN2", target_bir_lowering=False, debug=False)
    qry = nc.dram_tensor("qry", [Q, C, HW], F32, kind="ExternalInput").ap()
    sup = nc.dram_tensor("sup", [P, C, HW], F32, kind="ExternalInput").ap()
    oh = nc.dram_tensor("oh", [Q, P], F32, kind="ExternalInput").ap()
    ce = nc.dram_tensor("ce", [Q, 1], F32, kind="ExternalOutput").ap()
    cpd = nc.dram_tensor("cpd", [PN], F32).ap()
    rpd = nc.dram_tensor("rpd", [PN], F32).ap()
    gb2 = nc.dram_tensor("gb2", [NPART, F], BF16).ap()
    qd = nc.dram_tensor("qd", [2, QM], F32).ap()
    w1d = nc.dram_tensor("w1d", [P, QM], F32).ap()
    w2d = nc.dram_tensor("w2d", [Q, PN], F32).ap()
    zr = nc.dram_tensor("zr", [NPART, J], F32).ap()
    if DEBUG:
        emit.dbg = {
            "gp": nc.dram_tensor("dbg_gp", [NPART, F], BF16, kind="ExternalOutput").ap(),
            "sn": nc.dram_tensor("dbg_sn", [NPART, F], BF16, kind="ExternalOutput").ap(),
            "kn": nc.dram_tensor("dbg_kn", [NPART, F], BF16, kind="ExternalOutput").ap(),
            "km": nc.dram_tensor("dbg_km", [NPART, F], BF16, kind="ExternalOutput").ap(),
            "rq": nc.dram_tensor("dbg_rq", [NPART, HW * J], F32, kind="ExternalOutput").ap(),
            "at": nc.dram_tensor("dbg_at", [NPART, HW * J], F32, kind="ExternalOutput").ap(),
            "bt": nc.dram_tensor("dbg_bt", [NPART, HW * J], F32, kind="ExternalOutput").ap(),
            "su": nc.dram_tensor("dbg_su", [NPART, HW * J], F32, kind="ExternalOutput").ap(),
            "sv": nc.dram_tensor("dbg_sv", [NPART, HW * J], F32, kind="ExternalOutput").ap(),
            "uu": nc.dram_tensor("dbg_uu", [NPART, HW * J], BF16, kind="ExternalOutput").ap(),
            "vv": nc.dram_tensor("dbg_vv", [NPART, HW * J], BF16, kind="ExternalOutput").ap(),
            "zz": nc.dram_tensor("dbg_zz", [NPART, J], F32, kind="ExternalOutput").ap(),
            "ll": nc.dram_tensor("dbg_ll", [Q, P], F32, kind="ExternalOutput").ap(),
            "aqp": nc.dram_tensor("dbg_aqp", [NPART, HW * J], F32, kind="ExternalOutput").ap(),
            "sqp": nc.dram_tensor("dbg_sqp", [NPART, HW * J], F32, kind="ExternalOutput").ap(),
            "qd1": nc.dram_tensor("dbg_qd1", [QM], F32, kind="ExternalOutput").ap(),
        }
    with tile.TileContext(nc) as tc:
        emit(tc, qry, sup, oh, ce, cpd, rpd, gb2, qd, w1d, w2d, zr)
    nc.compile()
    return nc


def make_in_maps(support_xf, query_xf, query_y):
    support_xf = np.ascontiguousarray(np.asarray(support_xf, dtype=np.float32))
    query_xf = np.ascontiguousarray(np.asarray(query_xf, dtype=np.float32))
    query_y = np.asarray(query_y)
    in_maps = []
    for i in range(B):
        ohm = np.zeros((Q, P), np.float32)
        ohm[np.arange(Q), query_y[i].astype(np.int64)] = 1.0
        in_maps.append({
            "qry": query_xf[i].reshape(Q, C, HW),
            "sup": support_xf[i].reshape(P, C, HW),
            "oh": ohm,
        })
    return in_maps


def kernel(support_xf, query_xf, support_y, query_y, n_way=5, k_shot=1, **_):
    nc = build_program()
    in_maps = make_in_maps(support_xf, query_xf, query_y)
    res = run_bass_kernel_spmd(nc, in_maps, list(range(B)))
    ce = np.concatenate([res.results[i]["ce"].reshape(-1) for i in range(B)])
    return np.float32(ce.mean())
